# revision 12
# baseline (speedup 1.0000x reference)
"""GraphConv 2-layer GNN (GCN-style message passing) on 8 TRN2 NeuronCores.

Strategy (per the dst-partitioned sharding hint):
  - Nodes are degree-sorted and dealt round-robin to 8 cores (so every core's
    window w holds nodes of nearly-equal in-degree -> uniform gather padding).
  - Host ships per-core transposed features xT (features on partitions) so the
    PE contracts over F directly with zero on-device transposes.
  - Layer projections x@[w_rel|w_root] run on the PE (bias folded in as an
    extra all-ones feature row).
  - The 16-wide projected tables (h = x@w_rel1, h2 = h1@w_rel2) are AllGathered
    across cores into a replicated DRAM table; segment-sum aggregation is done
    per 128-dst-node window with one indirect row-gather DMA (padded CSR,
    sentinel zero row) + one strided DVE reduce.
  - ELU / log_softmax epilogues on DVE+ACT.

kernel(**inputs) takes the full unsharded inputs and returns the full
[N, 7] float32 log-softmax output.
"""

import os
import numpy as np

M = 8          # cores
P = 128        # partitions / dst-window size
TW = 16        # gather-table row width (floats) for BOTH layers (64B rows)
NODE_SUPER = 512  # node columns per projection superblock


def _plan(N, E, F, H, C):
    assert N % M == 0
    NCc = N // M                      # nodes per core
    W = (NCc + P - 1) // P            # dst windows per core
    NCPAD = W * P
    KROWS = F + 1                     # +1 all-ones bias row
    KCH = [128] * (KROWS // 128) + ([KROWS % 128] if KROWS % 128 else [])
    # superblocks of node columns (multiples of 128, <= NODE_SUPER)
    sblocks = []
    c0 = 0
    while c0 < NCPAD:
        w = min(NODE_SUPER, NCPAD - c0)
        sblocks.append((c0, w))
        c0 += w
    assert H <= TW and C <= TW
    return NCc, W, NCPAD, KCH, sblocks


def _host_preprocess(x, edge_index):
    """Returns per-core xtb/offs plus the permutation bookkeeping."""
    N, F = x.shape
    E = edge_index.shape[1]
    src_o = np.asarray(edge_index[0], dtype=np.int64)
    dst_o = np.asarray(edge_index[1], dtype=np.int64)

    deg = np.bincount(dst_o, minlength=N)
    order = np.argsort(-deg, kind="stable")          # rank -> old id
    deg_sorted = deg[order]
    NCc = N // M
    ranks = np.arange(N)
    perm = np.empty(N, np.int64)                     # old id -> new id
    perm[order] = (ranks % M) * NCc + ranks // M

    W = ((NCc + P - 1) // P)
    band_starts = np.minimum(np.arange(W) * (P * M), N - 1)
    deg_pad = np.maximum(deg_sorted[band_starts], 1).astype(np.int64)
    acc = np.zeros(W + 1, np.int64)
    acc[1:] = np.cumsum(deg_pad)
    D_pad = int(acc[-1])

    # bucket edges: new dst -> (core, window, partition, j-within-node)
    d_new = perm[dst_o]
    s_new = perm[src_o]
    eo = np.argsort(d_new, kind="stable")
    d_s = d_new[eo]
    s_s = s_new[eo]
    first = np.r_[True, d_s[1:] != d_s[:-1]]
    idx_first = np.flatnonzero(first)
    group_id = np.cumsum(first) - 1
    j = np.arange(E) - idx_first[group_id]
    core = d_s // NCc
    slot = d_s % NCc
    w = slot // P
    p = slot % P
    assert (j < deg_pad[w]).all(), "padding underestimate"
    col = acc[w] + j
    SENT = N
    offs = np.full((M, P, D_pad), SENT, np.int32)
    offs[core, p, col] = s_s.astype(np.int32)

    return order, perm, deg_pad, acc, D_pad, offs


def _build_program(N, F, H, C, D_pad, deg_pad, acc):
    import concourse.bass as bass
    import concourse.tile as tile
    from concourse import bacc, mybir
    from concourse.masks import make_identity

    f32 = mybir.dt.float32
    i32 = mybir.dt.int32
    AF = mybir.ActivationFunctionType
    OP = mybir.AluOpType

    NCc, W, NCPAD, KCH, sblocks = _plan(N, 0, F, H, C)
    H2 = 2 * H           # proj output width layer 1
    C2 = 2 * C           # proj output width layer 2

    nc = bacc.Bacc("TRN2", target_bir_lowering=False, debug=False)
    xtb = nc.declare_dram_parameter("xtb", [F + 1, NCPAD], f32, isOutput=False)
    offs_d = nc.declare_dram_parameter("offs", [P, D_pad], i32, isOutput=False)
    wc1 = nc.declare_dram_parameter("wc1", [F + 1, H2], f32, isOutput=False)
    wc2 = nc.declare_dram_parameter("wc2", [H + 1, C2], f32, isOutput=False)
    outp = nc.declare_dram_parameter("outp", [NCc, C], f32, isOutput=True)

    hloc = nc.dram_tensor("hloc", [NCc, TW], f32)
    h2loc = nc.dram_tensor("h2loc", [NCc, TW], f32)
    htab = nc.dram_tensor("htab", [N + 1, TW], f32, addr_space="Shared")
    h2tab = nc.dram_tensor("h2tab", [N + 1, TW], f32, addr_space="Shared")

    groups = [list(range(M))]

    with tile.TileContext(nc) as tc:
        with (
            tc.tile_pool(name="const", bufs=1) as cpool,
            tc.tile_pool(name="xw", bufs=3) as xpool,
            tc.tile_pool(name="pj", bufs=2, space="PSUM") as pjps,
            tc.tile_pool(name="sps", bufs=2, space="PSUM") as sps,
            tc.tile_pool(name="gath", bufs=3) as gpool,
            tc.tile_pool(name="work", bufs=3) as wpool,
        ):
            # ---- constants / persistent state ----
            ident = cpool.tile([P, P], f32)
            make_identity(nc, ident[:])
            ones_row = cpool.tile([1, P], f32)
            nc.vector.memset(ones_row[:], 1.0)
            zrow = cpool.tile([1, TW], f32)
            nc.vector.memset(zrow[:], 0.0)
            offs_sb = cpool.tile([P, D_pad], i32)
            nc.sync.dma_start(out=offs_sb[:], in_=offs_d[:])
            w1t = []
            ko = 0
            for i, K in enumerate(KCH):
                t = cpool.tile([K, H2], f32, tag=f"w1_{i}")
                nc.sync.dma_start(out=t[:], in_=wc1[ko:ko + K, :])
                w1t.append((t, ko, K))
                ko += K
            w2t = cpool.tile([H, C2], f32)
            nc.sync.dma_start(out=w2t[:], in_=wc2[0:H, :])
            w2b = cpool.tile([1, C2], f32)
            nc.sync.dma_start(out=w2b[:], in_=wc2[H:H + 1, :])
            r1buf = cpool.tile([P, W * H], f32)
            r2buf = cpool.tile([P, W * C], f32)
            # sentinel zero rows of the gather tables
            nc.gpsimd.dma_start(out=htab[N:N + 1, :], in_=zrow[:])
            nc.gpsimd.dma_start(out=h2tab[N:N + 1, :], in_=zrow[:])

            # ---- P1: projection h|r = x @ [w_rel1 | w_root1] (+bias row) ----
            # one 128-node block per PSUM tile: the accumulation group owns
            # its bank exclusively (interleaved groups sharing a bank clobber
            # each other on HW via start_tensor_calc's bank zeroing)
            for nb in range(W):
                ps = pjps.tile([P, H2], f32, tag="pj")
                for kc, (wt, ko, K) in enumerate(w1t):
                    xt = xpool.tile([128, P], f32, tag="xt")
                    nc.sync.dma_start(out=xt[:K, :],
                                      in_=xtb[ko:ko + K, nb * P:(nb + 1) * P])
                    nc.tensor.matmul(
                        out=ps[:],
                        lhsT=xt[:K, :],
                        rhs=wt[:],
                        start=(kc == 0),
                        stop=(kc == len(KCH) - 1),
                    )
                h_sb = wpool.tile([P, H2], f32, tag="hsb")
                nc.vector.tensor_copy(out=h_sb[:], in_=ps[:])
                nc.vector.tensor_copy(out=r1buf[:, nb * H:(nb + 1) * H],
                                      in_=h_sb[:, H:H2])
                nr = min(P, NCc - nb * P)
                if nr > 0:
                    nc.sync.dma_start(out=hloc[nb * P:nb * P + nr, 0:H],
                                      in_=h_sb[:nr, 0:H])

            # ---- P2: replicate h table ----
            tc.strict_bb_all_engine_barrier()
            nc.gpsimd.collective_compute(
                "AllGather", mybir.AluOpType.bypass, replica_groups=groups,
                ins=[hloc[:]], outs=[htab[0:N, :]],
            )
            tc.strict_bb_all_engine_barrier()

            # ---- P3: L1 aggregate + ELU + L2 projection ----
            # indirect DMA gathers one table row per partition per call
            # (HW consumes exactly one offset per dest partition-run)
            for w in range(W):
                dpw = int(deg_pad[w])
                a0 = int(acc[w])
                g = gpool.tile([P, dpw * TW], f32, tag="g")
                for j in range(dpw):
                    nc.gpsimd.indirect_dma_start(
                        out=g[:, j * TW:(j + 1) * TW], out_offset=None,
                        in_=htab[:],
                        in_offset=bass.IndirectOffsetOnAxis(
                            ap=offs_sb[:, a0 + j:a0 + j + 1], axis=0),
                    )
                agg = wpool.tile([P, TW], f32, tag="agg")
                nc.vector.tensor_reduce(
                    out=agg[:],
                    in_=g[:].rearrange("p (j f) -> p f j", f=TW),
                    axis=mybir.AxisListType.X, op=OP.add,
                )
                t = wpool.tile([P, H], f32, tag="t")
                nc.vector.tensor_add(out=t[:], in0=agg[:, 0:H],
                                     in1=r1buf[:, w * H:(w + 1) * H])
                # ELU(t) = relu(t) + min(exp(t)-1, 0)
                e = wpool.tile([P, H], f32, tag="e")
                nc.scalar.activation(out=e[:], in_=t[:], func=AF.Exp)
                ntl = wpool.tile([P, H], f32, tag="ntl")
                nc.vector.tensor_scalar(out=ntl[:], in0=e[:], scalar1=1.0,
                                        scalar2=0.0, op0=OP.subtract, op1=OP.min)
                pt = wpool.tile([P, H], f32, tag="pt")
                nc.scalar.activation(out=pt[:], in_=t[:], func=AF.Relu)
                h1 = wpool.tile([P, H], f32, tag="h1")
                nc.vector.tensor_add(out=h1[:], in0=pt[:], in1=ntl[:])
                # h1T for the L2 matmul
                h1t_ps = sps.tile([H, P], f32, tag="h1t")
                nc.tensor.transpose(out=h1t_ps[:], in_=h1[:], identity=ident[:])
                h1t = wpool.tile([H, P], f32, tag="h1ts")
                nc.scalar.copy(out=h1t[:], in_=h1t_ps[:])
                o2 = sps.tile([P, C2], f32, tag="o2")
                nc.tensor.matmul(out=o2[:], lhsT=h1t[:], rhs=w2t[:],
                                 start=True, stop=False)
                nc.tensor.matmul(out=o2[:], lhsT=ones_row[:],
                                 rhs=w2b[:], start=False, stop=True)
                o2s = wpool.tile([P, TW], f32, tag="o2s")
                nc.vector.tensor_copy(out=o2s[:, 0:C2], in_=o2[:])
                nc.vector.memset(o2s[:, C2:TW], 0.0)
                nr = min(P, NCc - w * P)
                nc.sync.dma_start(out=h2loc[w * P:w * P + nr, :],
                                  in_=o2s[:nr, :])
                nc.vector.tensor_copy(out=r2buf[:, w * C:(w + 1) * C],
                                      in_=o2s[:, C:C2])

            # ---- P4: replicate h2 table ----
            tc.strict_bb_all_engine_barrier()
            nc.gpsimd.collective_compute(
                "AllGather", mybir.AluOpType.bypass, replica_groups=groups,
                ins=[h2loc[:]], outs=[h2tab[0:N, :]],
            )
            tc.strict_bb_all_engine_barrier()

            # ---- P5: L2 aggregate + log_softmax ----
            for w in range(W):
                dpw = int(deg_pad[w])
                a0 = int(acc[w])
                g = gpool.tile([P, dpw * TW], f32, tag="g")
                for j in range(dpw):
                    nc.gpsimd.indirect_dma_start(
                        out=g[:, j * TW:(j + 1) * TW], out_offset=None,
                        in_=h2tab[:],
                        in_offset=bass.IndirectOffsetOnAxis(
                            ap=offs_sb[:, a0 + j:a0 + j + 1], axis=0),
                    )
                agg = wpool.tile([P, TW], f32, tag="agg")
                nc.vector.tensor_reduce(
                    out=agg[:],
                    in_=g[:].rearrange("p (j f) -> p f j", f=TW),
                    axis=mybir.AxisListType.X, op=OP.add,
                )
                lg = wpool.tile([P, C], f32, tag="lg")
                nc.vector.tensor_add(out=lg[:], in0=agg[:, 0:C],
                                     in1=r2buf[:, w * C:(w + 1) * C])
                mx = wpool.tile([P, 1], f32, tag="mx")
                nc.vector.tensor_reduce(out=mx[:], in_=lg[:],
                                        axis=mybir.AxisListType.X, op=OP.max)
                tt = wpool.tile([P, C], f32, tag="tt")
                nc.vector.tensor_tensor(out=tt[:], in0=lg[:],
                                        in1=mx[:].to_broadcast([P, C]),
                                        op=OP.subtract)
                ex = wpool.tile([P, C], f32, tag="ex")
                nc.scalar.activation(out=ex[:], in_=tt[:], func=AF.Exp)
                sm = wpool.tile([P, 1], f32, tag="sm")
                nc.vector.tensor_reduce(out=sm[:], in_=ex[:],
                                        axis=mybir.AxisListType.X, op=OP.add)
                ls = wpool.tile([P, 1], f32, tag="ls")
                nc.scalar.activation(out=ls[:], in_=sm[:], func=AF.Ln)
                res = wpool.tile([P, C], f32, tag="res")
                nc.vector.tensor_tensor(out=res[:], in0=tt[:],
                                        in1=ls[:].to_broadcast([P, C]),
                                        op=OP.subtract)
                nr = min(P, NCc - w * P)
                nc.sync.dma_start(out=outp[w * P:w * P + nr, :],
                                  in_=res[:nr, :])

    nc.compile()
    return nc


def _prepare(x, edge_index, w_root1, w_rel1, b_rel1, w_root2, w_rel2, b_rel2):
    x = np.ascontiguousarray(np.asarray(x, dtype=np.float32))
    edge_index = np.asarray(edge_index, dtype=np.int32)
    N, F = x.shape
    H = w_rel1.shape[1]
    C = w_rel2.shape[1]
    NCc, W, NCPAD, KCH, sblocks = _plan(N, edge_index.shape[1], F, H, C)

    order, perm, deg_pad, acc, D_pad, offs = _host_preprocess(x, edge_index)

    # per-core transposed features with trailing all-ones bias row
    xtb = np.zeros((M, F + 1, NCPAD), np.float32)
    for c in range(M):
        xtb[c, :F, :NCc] = x[order[c::M]].T
        xtb[c, F, :] = 1.0

    wc1 = np.zeros((F + 1, 2 * H), np.float32)
    wc1[:F, :H] = np.asarray(w_rel1, np.float32)
    wc1[:F, H:] = np.asarray(w_root1, np.float32)
    wc1[F, H:] = np.asarray(b_rel1, np.float32)

    wc2 = np.zeros((H + 1, 2 * C), np.float32)
    wc2[:H, :C] = np.asarray(w_rel2, np.float32)
    wc2[:H, C:] = np.asarray(w_root2, np.float32)
    wc2[H, C:] = np.asarray(b_rel2, np.float32)

    in_maps = [
        {"xtb": np.ascontiguousarray(xtb[c]),
         "offs": np.ascontiguousarray(offs[c]),
         "wc1": wc1, "wc2": wc2}
        for c in range(M)
    ]
    meta = dict(N=N, F=F, H=H, C=C, D_pad=D_pad, deg_pad=deg_pad, acc=acc,
                order=order)
    return in_maps, meta


def _assemble(results, meta):
    N, C = meta["N"], meta["C"]
    order = meta["order"]
    out = np.empty((N, C), np.float32)
    for c in range(M):
        out[order[c::M]] = results[c]["outp"]
    return out


def kernel(**inputs):
    from concourse.bass_utils import run_bass_kernel_spmd

    in_maps, meta = _prepare(**inputs)
    nc = _build_program(meta["N"], meta["F"], meta["H"], meta["C"],
                        meta["D_pad"], meta["deg_pad"], meta["acc"])
    trace = bool(int(os.environ.get("KERNEL_TRACE", "0")))
    res = run_bass_kernel_spmd(nc, in_maps, list(range(M)), trace=trace)
    if trace:
        kernel.last_exec_time_ns = res.exec_time_ns
    return _assemble(res.results, meta)


kernel.last_exec_time_ns = None
